# revision 16
# baseline (speedup 1.0000x reference)
"""Trainium2 Bass kernel for 12-head MHA (B=2, S=4096, D=768), fp32.

Sharding: 8 cores = 2 batches x 4 head-groups (3 heads each).
Each core computes, for its (batch, 3 heads):
    Q/K/V projections, scores^T = K @ Q^T (transposed-score layout),
    exp (ScalarE, fused 1/8 scale), AV with a ones-column appended to V
    (M=65 matmul -> softmax denominator lands in PSUM row 64 for free),
    normalize, and a partial out-projection  context @ Wo_slice^T.
Host sums the 4 partial outputs per batch and adds bo.

Input staging dominates the measured exec time (cost scales with input
bytes and tensor count), so each core receives ONE packed bf16 blob
[265, 4096] (2.17MB) holding only non-redundant data; full inputs are
rebuilt on-device with AllGathers:
  - rows 0:192: a distinct quarter of x[b]^T, AllGathered 4-way within
    the batch group (cores 4b..4b+3) to rebuild the full x^T,
  - row 192: packed q/k/v bias slices for this head-group,
  - rows 193:265: half of the packed weight block wpack [1024, 576]
    (wq^T | wk^T | wv^T side by side, then wo^T flattened), AllGathered
    2-way between the two cores sharing a head-group (c and c+4).

Matmul layouts keep the contraction dim on partitions:
  - Q^T duplicated on both partition halves so QK^T row-pairs two
    K-blocks (K=64 each) concurrently in the PE array,
  - K^T packed [128, 2048]: even S-blocks on partitions 0-63, odd on
    64-127 (built directly by gathered-rhs projection matmuls),
  - V natural [S,64] + ones col -> AV lhsT, exp tiles as AV rhs.
"""

import numpy as np

B, S, D = 2, 4096, 768
H, DK = 12, 64
NCORES = 8
HPC = 3                 # heads per core
DCH = D // 128          # 6 contraction chunks of 128
NT = S // 512           # 8 q-tiles / s-windows of 512
NKB = S // 128          # 32 key blocks of 128
GSZ = 2                 # k-blocks per exp group (2 PSUM banks, x2 buffers)
QR = D // 4             # 192 x^T rows per core quarter
WROWS = 72              # 512x576 weight half as 4096-wide rows
NROWS = QR + 1 + WROWS  # 265 blob rows

_CACHE = {}


def _build_bass(gather=True):
    from contextlib import ExitStack

    import concourse.bass as bass  # noqa: F401
    import concourse.mybir as mybir
    import concourse.tile as tile
    from concourse import bacc

    f32 = mybir.dt.float32
    Exp = mybir.ActivationFunctionType.Exp

    nc = bacc.Bacc("TRN2", target_bir_lowering=False, debug=False)
    nc.num_devices = NCORES
    bf16 = mybir.dt.bfloat16

    def mm(out, lhsT, rhs, **kw):
        nc.tensor.matmul(out, lhsT=lhsT, rhs=rhs, **kw)

    if gather:
        blob = nc.declare_dram_parameter("blob", [NROWS, S], bf16, isOutput=False)
    else:
        blob = nc.declare_dram_parameter(
            "blob", [D + 1 + 2 * WROWS, S], bf16, isOutput=False
        )
    out = nc.declare_dram_parameter("out", [S, D], f32, isOutput=True)

    with tile.TileContext(nc) as tc, ExitStack() as ctx:
        const = ctx.enter_context(tc.tile_pool(name="const", bufs=1))
        pdata = ctx.enter_context(tc.tile_pool(name="pdata", bufs=1))
        dram = ctx.enter_context(tc.tile_pool(name="dram", bufs=1, space="DRAM"))

        # ---- on-device input reconstruction (AllGathers) ----
        if gather:
            # x-gather first: it is the long pole; the small w-gather then
            # overlaps the xsb SBUF loads instead of delaying them.
            brow = QR + 1
            xb = dram.tile([QR, S], bf16, name="xb")
            xfull = dram.tile([D, S], bf16, name="xfull")
            wb = dram.tile([512, 576], bf16, name="wb")
            wfull = dram.tile([1024, 576], bf16, name="wfull")
            nc.gpsimd.dma_start(xb[:, :], blob[0:QR, :])
            nc.gpsimd.dma_start(
                wb[:, :],
                blob[brow : brow + WROWS, :]
                .rearrange("a c -> (a c)")
                .rearrange("(r k) -> r k", k=576),
            )
            nc.gpsimd.collective_compute(
                "AllGather",
                mybir.AluOpType.bypass,
                replica_groups=[[0, 1, 2, 3], [4, 5, 6, 7]],
                ins=[xb[:, :].opt()],
                outs=[xfull[:, :].opt()],
            )
            nc.gpsimd.collective_compute(
                "AllGather",
                mybir.AluOpType.bypass,
                replica_groups=[[0, 4], [1, 5], [2, 6], [3, 7]],
                ins=[wb[:, :].opt()],
                outs=[wfull[:, :].opt()],
            )
            bias_row = QR
        else:
            xfull = blob[0:D, :]
            wfull = (
                blob[D + 1 : D + 1 + 2 * WROWS, :]
                .rearrange("a c -> (a c)")
                .rearrange("(r k) -> r k", k=576)
            )
            bias_row = D
        wov = (
            wfull[768:1024, :]
            .rearrange("a c -> (a c)")
            .rearrange("(r k) -> r k", k=D)
        )  # [192, 768] wo^T

        ones = const.tile([1, 512], bf16, name="ones")
        nc.vector.memset(ones, 1.0)
        bq_sb = const.tile([1, HPC * DK], bf16, name="bq_sb")
        bk_sb = const.tile([1, HPC * DK], bf16, name="bk_sb")
        bv_sb = const.tile([1, HPC * DK], bf16, name="bv_sb")
        nc.sync.dma_start(out=bq_sb, in_=blob[bias_row : bias_row + 1, 0:192])
        nc.sync.dma_start(out=bk_sb, in_=blob[bias_row : bias_row + 1, 192:384])
        nc.sync.dma_start(out=bv_sb, in_=blob[bias_row : bias_row + 1, 384:576])

        # x resident in SBUF: 6 chunks of [128, S] (48KB/partition, bf16).
        # Loaded in column halves so early projection windows can start
        # before the whole of x has landed (tile tracks subtile deps).
        xsb = [const.tile([128, S], bf16, name=f"xsb{c}") for c in range(DCH)]
        for half in range(2):
            csl = slice(half * (S // 2), (half + 1) * (S // 2))
            for c in range(DCH):
                nc.sync.dma_start(
                    out=xsb[c][:, csl], in_=xfull[c * 128 : (c + 1) * 128, csl]
                )

        # Persistent per-head data.
        qdup = [
            [
                pdata.tile([128, 512], bf16, name=f"qd{h}_{t}", tag=f"qd{h}_{t}")
                for t in range(NT)
            ]
            for h in range(HPC)
        ]
        kt = [
            pdata.tile([128, NKB * 64], bf16, name=f"kt{h}", tag=f"kt{h}")
            for h in range(HPC)
        ]
        vaug = [
            pdata.tile([128, NKB, 65], bf16, name=f"va{h}", tag=f"va{h}")
            for h in range(HPC)
        ]
        ctxA = [
            pdata.tile([128, 512], bf16, name=f"ctxA{t}", tag=f"ctxA{t}")
            for t in range(NT)
        ]
        ctxB = [
            pdata.tile([64, 512], bf16, name=f"ctxB{t}", tag=f"ctxB{t}")
            for t in range(NT)
        ]

        for h in range(HPC):
            # ones column used by the AV denominator row
            nc.vector.memset(vaug[h][:, :, 64:65], 1.0)

        # ------- Phases 1+2 interleaved (shared pools) -------
        # Emission order: pass A (head-pair 0,1 projections + all V), then
        # per q-tile: attention for h0 interleaved with head-2 projections,
        # then h1, then h2 with the out-projection. This keeps ScalarE's
        # exp stream (the phase-2 bottleneck) starting ~60us earlier while
        # head-2 projection matmuls fill TensorE's idle slots.
        # PSUM (8 banks): scores 2x2 (pass-A Q reuses), av 1x2 (pass-A K
        # reuses), po1 1 + po2 1 (pass-A V and pass-B Q/K reuse).
        with (
            tc.tile_pool(name="ph1", bufs=1) as ph1,
            tc.tile_pool(name="ph2", bufs=1) as ph2,
            tc.tile_pool(name="php", bufs=1, space="PSUM") as php,
            tc.tile_pool(name="rcdp", bufs=2, space="DRAM") as rcdp,
        ):
            wq_sb = ph1.tile([128, DCH, HPC * DK], bf16, name="wq_sb")
            wk_sb = ph1.tile([128, DCH, HPC * DK], bf16, name="wk_sb")
            wv_sb = ph1.tile([128, DCH, HPC * DK], bf16, name="wv_sb")
            for i, wsb in enumerate((wq_sb, wk_sb, wv_sb)):
                nc.sync.dma_start(
                    out=wsb,
                    in_=wfull[0:768, i * 192 : (i + 1) * 192].rearrange(
                        "(c p) m -> p c m", p=128
                    ),
                )
            wo_a = ph2.tile([128, D], bf16, name="wo_a")
            wo_b = ph2.tile([64, D], bf16, name="wo_b")
            nc.sync.dma_start(out=wo_a, in_=wov[0:128, :])
            nc.sync.dma_start(out=wo_b, in_=wov[128:192, :])

            def proj_window(w, h0, mw, qtag, qbufs, ktag, kbufs, act_copies):
                """Q/K projections for one 512-col window of head-group h0."""
                wsl = slice(w * 512, (w + 1) * 512)
                hh_list = [h0, h0 + 1] if mw == 128 else [h0]
                hsl = slice(h0 * DK, h0 * DK + mw)
                cp2 = nc.scalar.copy if act_copies else nc.vector.tensor_copy
                # ---- Q^T, then duplicate into both partition halves ----
                pq = php.tile(
                    [128, 512], f32, name=f"pq{w}_{h0}", tag=qtag, bufs=qbufs
                )
                for c in range(DCH):
                    mm(pq[0:mw, :], lhsT=wq_sb[:, c, hsl], rhs=xsb[c][:, wsl],
                       start=(c == 0), stop=False)
                mm(pq[0:mw, :], lhsT=bq_sb[:, hsl], rhs=ones[:, :],
                   start=False, stop=True)
                for hh in hh_list:
                    r0 = (hh - h0) * 64
                    nc.vector.tensor_copy(
                        qdup[hh][w][0:64, :], pq[r0 : r0 + 64, :]
                    )
                    cp2(qdup[hh][w][64:128, :], pq[r0 : r0 + 64, :])

                # ---- K^T, one natural N=512 chain; the even/odd key-block
                # packing (even -> partitions 0-63, odd -> 64-127) happens in
                # the split copies below (a bank allows only one accumulation
                # group at a time) ----
                pk = php.tile(
                    [128, 512], f32, name=f"pk{w}_{h0}", tag=ktag, bufs=kbufs
                )
                for c in range(DCH):
                    mm(pk[0:mw, :], lhsT=wk_sb[:, c, hsl], rhs=xsb[c][:, wsl],
                       start=(c == 0), stop=False)
                mm(pk[0:mw, :], lhsT=bk_sb[:, hsl], rhs=ones[:, :],
                   start=False, stop=True)
                for hh in hh_list:
                    r0 = (hh - h0) * 64
                    for blk in range(2):
                        c0 = w * 256 + blk * 128
                        nc.vector.tensor_copy(
                            kt[hh][0:64, c0 : c0 + 128],
                            pk[r0 : r0 + 64, blk * 256 : blk * 256 + 128],
                        )
                        cp2(
                            kt[hh][64:128, c0 : c0 + 128],
                            pk[r0 : r0 + 64, blk * 256 + 128 : blk * 256 + 256],
                        )

            def proj_v_window(w):
                """V projections (all 3 heads) for one window."""
                for sc in range(4):
                    j = w * 4 + sc
                    pv = php.tile(
                        [128, 512], f32, name=f"pv{w}_{sc}",
                        tag=("po1", "po2")[sc % 2], bufs=1,
                    )
                    ssl = slice(w * 512 + sc * 128, w * 512 + (sc + 1) * 128)
                    for c in range(DCH):
                        mm(
                            pv[:, 0 : HPC * DK], lhsT=xsb[c][:, ssl],
                            rhs=wv_sb[:, c, :], start=(c == 0), stop=False,
                        )
                    mm(
                        pv[:, 0 : HPC * DK], lhsT=ones[:, 0:128], rhs=bv_sb,
                        start=False, stop=True,
                    )
                    for h in range(HPC):
                        if h == 1:
                            nc.scalar.copy(
                                vaug[h][:, j, 0:64],
                                pv[:, h * DK : (h + 1) * DK],
                            )
                        else:
                            nc.vector.tensor_copy(
                                vaug[h][:, j, 0:64],
                                pv[:, h * DK : (h + 1) * DK],
                            )

            def attn(t, h):
                """Attention for one (q-tile, head): scores, exp, AV, norm."""
                pav = php.tile(
                    [65, 512], f32, name=f"av{t}_{h}", tag="av", bufs=2
                )
                for g0 in range(0, NKB, GSZ):
                    blocks = list(range(g0, g0 + GSZ))
                    ps = php.tile(
                        [128, GSZ * 512], f32,
                        name=f"sc{t}_{h}_{g0}", tag="scores", bufs=2,
                    )
                    for i, j in enumerate(blocks):
                        pb = (j % 2) * 64
                        col0 = (j // 4) * 256 + ((j % 4) // 2) * 128
                        mm(
                            ps[:, i * 512 : (i + 1) * 512],
                            lhsT=kt[h][pb : pb + 64, col0 : col0 + 128],
                            rhs=qdup[h][t][pb : pb + 64, :],
                            start=True, stop=True,
                        )
                    et = ph2.tile(
                        [128, GSZ * 512], bf16,
                        name=f"et{t}_{h}_{g0}", tag="et", bufs=2,
                    )
                    nc.scalar.activation(et, ps, Exp, scale=0.125)
                    for i, j in enumerate(blocks):
                        mm(
                            pav,
                            lhsT=vaug[h][:, j, :],
                            rhs=et[:, i * 512 : (i + 1) * 512],
                            start=(j == 0), stop=(j == NKB - 1),
                        )
                # normalize: recip of denominator row, broadcast, multiply
                rc = ph2.tile([65, 512], f32, name=f"rc{t}_{h}", tag="rc", bufs=2)
                nc.vector.reciprocal(rc[64:65, :], pav[64:65, :])
                rcd = rcdp.tile([1, 512], f32, name=f"rcd{t}_{h}", tag="rcd")
                nc.sync.dma_start(out=rcd, in_=rc[64:65, :])
                bc = ph2.tile([64, 512], f32, name=f"bc{t}_{h}", tag="bc", bufs=2)
                nc.sync.dma_start(out=bc, in_=rcd.partition_broadcast(64))
                if h == 0:
                    dst = ctxA[t][0:64, :]
                elif h == 1:
                    dst = ctxA[t][64:128, :]
                else:
                    dst = ctxB[t][0:64, :]
                nc.vector.tensor_mul(dst, pav[0:64, :], bc)

            def outproj(t):
                """Partial out-projection for q-tile t (all 3 local heads)."""
                for sci in range(4):
                    scn = t * 4 + sci
                    ssl = slice(scn * 128, (scn + 1) * 128)
                    csl = slice(sci * 128, (sci + 1) * 128)
                    po1 = php.tile(
                        [128, 512], f32, name=f"po1_{scn}", tag="po1", bufs=1
                    )
                    po2 = php.tile(
                        [128, 512], f32, name=f"po2_{scn}", tag="po2", bufs=1
                    )
                    mm(po1, lhsT=ctxA[t][:, csl], rhs=wo_a[:, 0:512],
                       start=True, stop=False)
                    mm(po1, lhsT=ctxB[t][:, csl], rhs=wo_b[:, 0:512],
                       start=False, stop=True)
                    mm(po2[:, 0:256], lhsT=ctxA[t][:, csl], rhs=wo_a[:, 512:768],
                       start=True, stop=False)
                    mm(po2[:, 0:256], lhsT=ctxB[t][:, csl], rhs=wo_b[:, 512:768],
                       start=False, stop=True)
                    ot = ph2.tile([128, D], f32, name=f"ot{scn}", tag="ot", bufs=3)
                    nc.vector.tensor_copy(ot[:, 0:512], po1)
                    nc.vector.tensor_copy(ot[:, 512:768], po2[:, 0:256])
                    nc.sync.dma_start(out=out[ssl, :], in_=ot)

            # pass A: head-pair (0,1) projections + all V (ACT still idle)
            for w in range(NT):
                proj_window(w, 0, 128, "scores", 2, "av", 2, act_copies=True)
                proj_v_window(w)
            # h0 attention, head-2 projections woven into TensorE idle slots
            for t in range(NT):
                attn(t, 0)
                proj_window(t, 2, 64, "po1", 1, "po2", 1, act_copies=False)
            for t in range(NT):
                attn(t, 1)
            # emit outproj one tile late so its ctx dependencies (which
            # include the normalize DMA roundtrip) are satisfied before it
            # reaches the head of the in-order PE queue -- otherwise exp of
            # the next tile stalls ~1.6us behind it at every tile boundary.
            for t in range(NT):
                attn(t, 2)
                if t > 0:
                    outproj(t - 1)
            outproj(NT - 1)

    nc.compile()
    return nc


def _get_nc():
    if "nc" not in _CACHE:
        _CACHE["nc"] = _build_bass()
    return _CACHE["nc"]


def _get_nc_nogather():
    if "ncng" not in _CACHE:
        _CACHE["ncng"] = _build_bass(gather=False)
    return _CACHE["ncng"]


def _packs(x, Wq, bq, Wk, bk, Wv, bv, Wo):
    """Host-side shared prep: x^T per batch (bf16) and wpack per head-group."""
    import ml_dtypes

    bf = ml_dtypes.bfloat16
    x = np.asarray(x, dtype=np.float32)
    xhb = [np.ascontiguousarray(x[b].T).astype(bf) for b in range(B)]  # [D, S]
    wpacks = []
    for hg in range(4):
        rows = slice(hg * HPC * DK, (hg + 1) * HPC * DK)
        wqT = np.asarray(Wq, np.float32)[rows, :].T.astype(bf)      # [D, 192]
        wkT = np.asarray(Wk, np.float32)[rows, :].T.astype(bf)
        wvT = np.asarray(Wv, np.float32)[rows, :].T.astype(bf)
        woT = np.asarray(Wo, np.float32)[:, rows].T.astype(bf)      # [192, D]
        wp = np.concatenate(
            [
                np.concatenate([wqT, wkT, wvT], axis=1),   # [768, 576]
                woT.reshape(256, 576),                     # [256, 576]
            ],
            axis=0,
        )  # [1024, 576]
        wpacks.append(wp)
    bqs = np.asarray(bq, np.float32).astype(bf)
    bks = np.asarray(bk, np.float32).astype(bf)
    bvs = np.asarray(bv, np.float32).astype(bf)
    return xhb, wpacks, bqs, bks, bvs


def make_in_maps(x, Wq, bq, Wk, bk, Wv, bv, Wo, bo):
    """Per-core minimized single-blob inputs (x quarter + biases + W half)."""
    import ml_dtypes

    bf = ml_dtypes.bfloat16
    xhb, wpacks, bqs, bks, bvs = _packs(x, Wq, bq, Wk, bk, Wv, bv, Wo)
    in_maps = []
    for c in range(NCORES):
        b, hg, half = c // 4, c % 4, c // 4
        blob = np.zeros((NROWS, S), dtype=bf)
        blob[0:QR] = xhb[b][hg * QR : (hg + 1) * QR, :]
        blob[QR, 0:192] = bqs[hg * 192 : (hg + 1) * 192]
        blob[QR, 192:384] = bks[hg * 192 : (hg + 1) * 192]
        blob[QR, 384:576] = bvs[hg * 192 : (hg + 1) * 192]
        blob[QR + 1 :] = wpacks[hg][half * 512 : (half + 1) * 512, :].reshape(
            WROWS, S
        )
        in_maps.append({"blob": blob})
    return in_maps


def make_in_maps_nogather(x, Wq, bq, Wk, bk, Wv, bv, Wo, bo):
    """Full (ungathered) per-core inputs for the no-collective build."""
    import ml_dtypes

    bf = ml_dtypes.bfloat16
    xhb, wpacks, bqs, bks, bvs = _packs(x, Wq, bq, Wk, bk, Wv, bv, Wo)
    in_maps = []
    for c in range(NCORES):
        b, hg = c // 4, c % 4
        blob = np.zeros((D + 1 + 2 * WROWS, S), dtype=bf)
        blob[0:D] = xhb[b]
        blob[D, 0:192] = bqs[hg * 192 : (hg + 1) * 192]
        blob[D, 192:384] = bks[hg * 192 : (hg + 1) * 192]
        blob[D, 384:576] = bvs[hg * 192 : (hg + 1) * 192]
        blob[D + 1 :] = wpacks[hg].reshape(2 * WROWS, S)
        in_maps.append({"blob": blob})
    return in_maps


def kernel(x, Wq, bq, Wk, bk, Wv, bv, Wo, bo, _trace=False):
    from concourse.bass_utils import run_bass_kernel_spmd

    nc = _get_nc()
    in_maps = make_in_maps(x, Wq, bq, Wk, bk, Wv, bv, Wo, bo)
    res = run_bass_kernel_spmd(
        nc, in_maps, core_ids=list(range(NCORES)), trace=_trace
    )
    _CACHE["last_results"] = res
    out = np.zeros((B, S, D), dtype=np.float32)
    for c in range(NCORES):
        out[c // 4] += res.results[c]["out"]
    out += np.asarray(bo, dtype=np.float32)[None, None, :]
    return out
